# revision 33
# baseline (speedup 1.0000x reference)
"""Trainium2 Bass kernel: MEGNet GlobalModel (graph aggregation + 3-layer MLP w/ BatchNorm).

Strategy (graph-parallel over 8 NeuronCores):
  - 2048 graphs -> 64 windows of 32 graphs; core c owns windows 8c..8c+7.
  - Host folds the two chained scatter_means into the DATA itself:
        ea'[e] = edge_attr[e] * (1/max(deg[src_e],1)) * (1/max(cnt[g],1)) * S_e
        x'[n]  = x[n] * (1/max(cnt[g],1)) * S_x
    (S_* power-of-two scales keeping fp8 in range; undone at PSUM evacuation),
    sorts edges by graph id, and packs both streams chunk-major in fp8e4m3.
  - Device: per 256-row (2-tile) group, a 0/1 selection matrix sel[i, j] =
    (local_gid_i == j) covering the FULL 32-graph window is matmul'ed against
    the data in ONE DoubleRow fp8 matmul (2 edges per PE cell), accumulating
    per-graph sums in a [32, D] PSUM acc. Because sel spans the whole window,
    no slot bases / fixups / metadata are needed; the first pair of each
    window sets start=True which also zero-fills rows of empty graphs.
  - MLP is graph-sharded: each core runs the 3-layer MLP only for its own 256
    graphs in [feature, graph] layout. BatchNorm batch stats are the only
    cross-core coupling: per layer, a tiny [128, 4] f32 AllReduce of
    (sum, sumsq). A dummy warm-up collective at kernel start hides the cc
    firmware's first-launch latency. No AllGather of activations at all;
    the host concatenates the 8 per-core [D, 256] outputs.
"""

import sys

sys.path.insert(0, "/opt/trn_rl_repo")

import ml_dtypes
import numpy as np

from concourse import bacc, bass, bass_utils, mybir, tile
from concourse.masks import make_identity

F32 = mybir.dt.float32
F16 = mybir.dt.float16
F8 = mybir.dt.float8e4
NPF8 = ml_dtypes.float8_e4m3
P = 128
EPS = 1e-5
NCORES = 8
WIN = 32  # graphs per window (= sel width)
NWPC = 8  # windows per core
CHE = 32  # edge-stream [128, D] tiles per DMA chunk (1 MB chunks)
CHX = 16  # x-stream tiles per chunk
DR = mybir.MatmulPerfMode.DoubleRow
ALU = mybir.AluOpType
ACTF = mybir.ActivationFunctionType
AX = mybir.AxisListType
USE_TTR = False  # DVE tensor_tensor_reduce hangs the device on this stack

_prog_cache: dict = {}


def _ceil_to(a: int, m: int) -> int:
    return -(-a // m) * m


# ---------------------------------------------------------------- device program


def _emit(nc, tc, cfg, ap):
    D, NG, EW, XW = cfg["D"], cfg["NG"], cfg["EW"], cfg["XW"]
    nt_e, nt_x = EW // P, XW // P
    DJ = D // P  # feature tiles per 128 partitions (2)
    K1 = 3 * D // P  # k-tiles of layer 1 (6)
    GPC = NG // NCORES  # graphs per core (256)
    assert NG == NCORES * NWPC * WIN and D % P == 0

    with (
        tc.tile_pool(name="const", bufs=1) as cpool,
        tc.tile_pool(name="tables", bufs=1) as tpool,
        tc.tile_pool(name="data", bufs=6) as dpool,
        tc.tile_pool(name="eq", bufs=4) as qpool,
        tc.tile_pool(name="evac", bufs=2) as epool,
        tc.tile_pool(name="psum", bufs=2, space="PSUM") as ppool,
        tc.tile_pool(name="psumL1", bufs=1, space="PSUM") as pl1pool,
        tc.tile_pool(name="mlp", bufs=1) as mpool,
        tc.tile_pool(name="stats", bufs=2) as stpool,
        tc.tile_pool(name="dram", bufs=1, space="DRAM") as drpool,
    ):
        # --- constants
        ident = cpool.tile([P, P], F32)
        make_identity(nc, ident[:])
        iota_i = cpool.tile([P, WIN], mybir.dt.int32)
        nc.gpsimd.iota(iota_i[:], pattern=[[1, WIN]], base=0, channel_multiplier=0)
        iota16 = cpool.tile([P, WIN], F16)
        nc.vector.tensor_copy(iota16[:], iota_i[:])
        eps_sb = cpool.tile([P, 1], F32)
        nc.vector.memset(eps_sb[:], EPS)
        zero_sb = cpool.tile([P, 1], F32)
        nc.vector.memset(zero_sb[:], 0.0)
        iota3 = iota16[:, :].rearrange("p (o f) -> p o f", o=1)

        # --- per-row tables: window-local gid per tile column
        def table(name, cols):
            t = tpool.tile([P, cols], F16, name=name)
            nc.scalar.dma_start(t[:], ap[name][:, :])
            return t

        eg16 = table("eg16", NWPC * nt_e)
        xg16 = table("xg16", NWPC * nt_x)

        # --- MLP params, prefetched up front
        w0_sb = mpool.tile([P, K1, D], F16)
        nc.scalar.dma_start(w0_sb[:], ap["w0t"][:, :].rearrange("(a p) f -> p a f", p=P))
        w1_sb = mpool.tile([P, DJ, D], F16)
        w2_sb = mpool.tile([P, DJ, D], F16)
        par_sb = mpool.tile([P, DJ, 9], F32)
        nc.scalar.dma_start(par_sb[:], ap["par"][:, :].rearrange("(a p) c -> p a c", p=P))
        ut_sb = mpool.tile([P, DJ, GPC], F16)
        nc.scalar.dma_start(ut_sb[:], ap["ut"][:, :].rearrange("(a p) g -> p a g", p=P))

        # --- warm-up collectives (issued mid-stream; see main flow)
        warm_in = drpool.tile([P, 1], F32, name="warm_in")
        warm_outs = [
            drpool.tile([P, 1], F32, addr_space="Shared", name=f"warm_out{i}")
            for i in range(1)
        ]
        nc.sync.dma_start(warm_in[:], zero_sb[:])

        def warmup_collective(i):
            nc.gpsimd.collective_compute(
                "AllReduce",
                ALU.add,
                replica_groups=[list(range(NCORES))],
                ins=[warm_in.opt()],
                outs=[warm_outs[i].opt()],
            )

        # comb^T k-tiles per window: [ue0, ue1, uv0, uv1]; u comes from ut_sb
        combT = mpool.tile([P, NWPC, 4, WIN], F16, name="combT")

        # --- one window of segment-sum: acc[g, :] += sel.T @ rows (DoubleRow fp8)
        # chunk schedule: full CH-tile chunks + one per-window remainder chunk
        def seg_window(full_ap, rem_ap, g16, nt, win, inv_s, dst_k0, ch):
            acc = ppool.tile([WIN, 512], F32, tag="acc")  # full PSUM bank
            npair = nt // 2
            nfull, rem = nt // ch, nt % ch
            sched = [(full_ap, (win * nfull + c) * P, c * ch, ch) for c in range(nfull)]
            if rem:
                sched.append((rem_ap, win * P, nfull * ch, rem))
            for c, (src, r0, tile0, cw) in enumerate(sched):
                chunk = dpool.tile([P, cw, D], F8, tag="data")
                # alternate the two HWDGE rings to double descriptor throughput
                eng = nc.sync if c % 2 == 0 else nc.scalar
                eng.dma_start(chunk[:], src[r0 : r0 + P, :])
                cl, cr = win * nt + tile0, win * nt + tile0 + cw
                eq = qpool.tile([P, cw, WIN], F8, tag="eq")
                nc.vector.tensor_tensor(
                    out=eq[:],
                    in0=iota3.to_broadcast([P, cw, WIN]),
                    in1=g16[:, cl:cr].rearrange("p (c o) -> p c o", o=1).to_broadcast(
                        [P, cw, WIN]
                    ),
                    op=ALU.is_equal,
                )
                for s in range(cw // 2):
                    pr = tile0 // 2 + s
                    nc.tensor.matmul(
                        acc[:, 0:D],
                        lhsT=eq[:, 2 * s : 2 * s + 2, :],
                        rhs=chunk[:, 2 * s : 2 * s + 2, :],
                        start=(pr == 0),
                        stop=(pr == npair - 1),
                        perf_mode=DR,
                        skip_group_check=True,
                    )
            # evacuate: descale, transpose [WIN, 128] blocks into comb^T k-tiles
            acc_sb = epool.tile([WIN, D], F32, tag="acc_sb")
            nc.scalar.activation(acc_sb[:], acc[:, 0:D], ACTF.Copy, scale=inv_s)
            for fh in range(DJ):
                tp = ppool.tile([P, 512], F32, tag="mm")  # full PSUM bank
                nc.tensor.transpose(
                    tp[:, 0:WIN], acc_sb[:, fh * P : (fh + 1) * P], ident[0:WIN, 0:WIN]
                )
                nc.scalar.copy(combT[:, win, dst_k0 + fh, :], tp[:, 0:WIN])

        # --- graph-sharded MLP pieces
        psL1 = pl1pool.tile([P, DJ, NWPC, WIN], F32, name="psL1")
        h0 = mpool.tile([P, DJ, GPC], F16, name="h0")
        h1 = mpool.tile([P, DJ, GPC], F16, name="h1")
        hout = mpool.tile([P, DJ, GPC], F32, name="hout")
        st0 = mpool.tile([P, 2, DJ, NWPC], F32, name="st0")  # L1 (sum, sq) per window

        def l1_window(win):
            for jt in range(DJ):
                for kk in range(K1):
                    rhs = (
                        combT[:, win, kk, :]
                        if kk < 4
                        else ut_sb[:, kk - 4, win * WIN : (win + 1) * WIN]
                    )
                    nc.tensor.matmul(
                        psL1[:, jt, win, :],
                        lhsT=w0_sb[:, kk, jt * P : (jt + 1) * P],
                        rhs=rhs,
                        start=(kk == 0),
                        stop=(kk == K1 - 1),
                    )
                hsl = h0[:, jt, win * WIN : (win + 1) * WIN]
                nc.scalar.activation(
                    hsl,
                    psL1[:, jt, win, :],
                    ACTF.Relu,
                    bias=par_sb[:, jt, 0:1],
                    scale=1.0,
                    accum_out=st0[:, 0, jt, win : win + 1],
                )
                scr = stpool.tile([P, WIN], F16, tag="scr")
                if USE_TTR:
                    nc.vector.tensor_tensor_reduce(
                        out=scr[:], in0=hsl, in1=hsl, scale=1.0, scalar=0.0,
                        op0=ALU.mult, op1=ALU.add,
                        accum_out=st0[:, 1, jt, win : win + 1],
                    )
                else:
                    nc.scalar.activation(
                        scr[:], hsl, ACTF.Square, scale=1.0,
                        accum_out=st0[:, 1, jt, win : win + 1],
                    )

        def bn_sync(layer, st_in):
            # st_in: [P, 2, DJ] f32 local (sum, sumsq) -> global scl/bv
            sdr = drpool.tile([P, 2 * DJ], F32, name=f"stat{layer}")
            gdr = drpool.tile([P, 2 * DJ], F32, addr_space="Shared", name=f"gstat{layer}")
            nc.sync.dma_start(sdr[:], st_in[:].rearrange("p a b -> p (a b)"))
            nc.gpsimd.collective_compute(
                "AllReduce",
                ALU.add,
                replica_groups=[list(range(NCORES))],
                ins=[sdr.opt()],
                outs=[gdr.opt()],
            )
            st_g = stpool.tile([P, 2, DJ], F32, tag="stg")
            nc.scalar.dma_start(st_g[:], gdr[:, :].rearrange("p (a b) -> p a b", b=DJ))
            me = stpool.tile([P, 2, DJ], F32, tag="me")
            nc.vector.tensor_scalar_mul(me[:], st_g[:], 1.0 / NG)
            mean, esq = me[:, 0, :], me[:, 1, :]
            m2 = stpool.tile([P, DJ], F32, tag="m2")
            nc.vector.tensor_tensor(out=m2[:], in0=mean, in1=mean, op=ALU.mult)
            var = stpool.tile([P, DJ], F32, tag="var")
            nc.vector.tensor_tensor(out=var[:], in0=esq, in1=m2[:], op=ALU.subtract)
            std = stpool.tile([P, DJ], F32, tag="std")
            nc.scalar.activation(std[:], var[:], ACTF.Sqrt, bias=eps_sb[:], scale=1.0)
            rstd = stpool.tile([P, DJ], F32, tag="rstd")
            nc.vector.reciprocal(rstd[:], std[:])
            scl = stpool.tile([P, DJ], F32, tag="scl")
            nc.vector.tensor_tensor(
                out=scl[:], in0=rstd[:], in1=par_sb[:, :, 3 + layer], op=ALU.mult
            )
            mscl = stpool.tile([P, DJ], F32, tag="mscl")
            nc.vector.tensor_tensor(out=mscl[:], in0=mean, in1=scl[:], op=ALU.mult)
            bv = stpool.tile([P, DJ], F32, tag="bv")
            nc.vector.tensor_tensor(
                out=bv[:], in0=par_sb[:, :, 6 + layer], in1=mscl[:], op=ALU.subtract
            )
            return scl, bv

        def apply_bn(h, scl, bv):
            for jt in range(DJ):
                nc.vector.tensor_scalar(
                    h[:, jt, :],
                    h[:, jt, :],
                    scalar1=scl[:, jt : jt + 1],
                    scalar2=bv[:, jt : jt + 1],
                    op0=ALU.mult,
                    op1=ALU.add,
                )

        def mid_layer(layer, w_sb, h_in, h_out):
            st_in = stpool.tile([P, 2, DJ], F32, tag="stin")
            ps = ppool.tile([P, DJ, GPC], F32, tag="mlp")
            for jt in range(DJ):
                for kk in range(DJ):
                    nc.tensor.matmul(
                        ps[:, jt, :],
                        lhsT=w_sb[:, kk, jt * P : (jt + 1) * P],
                        rhs=h_in[:, kk, :],
                        start=(kk == 0),
                        stop=(kk == DJ - 1),
                    )
                nc.scalar.activation(
                    h_out[:, jt, :],
                    ps[:, jt, :],
                    ACTF.Relu,
                    bias=par_sb[:, jt, layer : layer + 1],
                    scale=1.0,
                    accum_out=st_in[:, 0, jt : jt + 1],
                )
                scr = stpool.tile([P, GPC], F16, tag="scr")
                if USE_TTR:
                    nc.vector.tensor_tensor_reduce(
                        out=scr[:], in0=h_out[:, jt, :], in1=h_out[:, jt, :],
                        scale=1.0, scalar=0.0, op0=ALU.mult, op1=ALU.add,
                        accum_out=st_in[:, 1, jt : jt + 1],
                    )
                else:
                    nc.scalar.activation(
                        scr[:], h_out[:, jt, :], ACTF.Square, scale=1.0,
                        accum_out=st_in[:, 1, jt : jt + 1],
                    )
            return st_in

        # --- main flow
        for win in range(NWPC):
            seg_window(
                ap.get("xa"), ap.get("xar"), xg16, nt_x, win,
                1.0 / cfg["sx"], 2, CHX,
            )
            seg_window(
                ap.get("ea"), ap.get("ear"), eg16, nt_e, win,
                1.0 / cfg["se"], 0, CHE,
            )
            l1_window(win)
            if win == 0:
                # prime the collectives firmware so the BN AllReduces start hot
                warmup_collective(0)
                # L2/L3 weights aren't needed until the tail; load them behind
                # the first window's chunks so they don't delay the stream
                nc.scalar.dma_start(
                    w1_sb[:], ap["w1t"][:, :].rearrange("(a p) f -> p a f", p=P)
                )
                nc.scalar.dma_start(
                    w2_sb[:], ap["w2t"][:, :].rearrange("(a p) f -> p a f", p=P)
                )

        st_l1 = stpool.tile([P, 2, DJ], F32, tag="stin")
        nc.vector.tensor_reduce(st_l1[:], st0[:], axis=AX.X, op=ALU.add)
        scl, bv = bn_sync(0, st_l1)
        apply_bn(h0, scl, bv)
        st_l2 = mid_layer(1, w1_sb, h0, h1)
        scl, bv = bn_sync(1, st_l2)
        apply_bn(h1, scl, bv)
        st_l3 = mid_layer(2, w2_sb, h1, hout)
        scl, bv = bn_sync(2, st_l3)
        apply_bn(hout, scl, bv)

        for jt in range(DJ):
            nc.sync.dma_start(ap["out_t"][jt * P : (jt + 1) * P, :], hout[:, jt, :])


def _build_program(cfg):
    key = repr(sorted(cfg.items(), key=lambda kv: kv[0]))
    if key in _prog_cache:
        return _prog_cache[key]
    D, NG, EW, XW = cfg["D"], cfg["NG"], cfg["EW"], cfg["XW"]
    nt_e, nt_x = EW // P, XW // P
    GPC = NG // NCORES
    nc = bacc.Bacc(
        "TRN2",
        target_bir_lowering=False,
        debug=False,
        enable_asserts=False,
        num_devices=NCORES,
    )
    ap = {}
    ins = [
        ("eg16", [P, NWPC * nt_e], F16),
        ("xg16", [P, NWPC * nt_x], F16),
        ("ut", [D, GPC], F16),
        ("w0t", [3 * D, D], F16),
        ("w1t", [D, D], F16),
        ("w2t", [D, D], F16),
        ("par", [D, 9], F32),
    ]
    for nt, ch, full, remn in ((nt_e, CHE, "ea", "ear"), (nt_x, CHX, "xa", "xar")):
        nf, rem = nt // ch, nt % ch
        if nf:
            ins.append((full, [NWPC * nf * P, ch * D], F8))
        if rem:
            ins.append((remn, [NWPC * P, rem * D], F8))
    for name, shape, dt in ins:
        ap[name] = nc.dram_tensor(name, shape, dt, kind="ExternalInput").ap()
    ap["out_t"] = nc.dram_tensor("out_t", [D, GPC], F32, kind="ExternalOutput").ap()

    with tile.TileContext(nc) as tc:
        _emit(nc, tc, cfg, ap)
    nc.compile()
    _prog_cache[key] = nc
    return nc


# ---------------------------------------------------------------- host side


def _pow2_scale(v: np.ndarray) -> float:
    m = float(np.max(np.abs(v))) if v.size else 0.0
    if not np.isfinite(m) or m <= 0.0:
        return 1.0
    s = 2.0 ** np.floor(np.log2(224.0 / m))
    return float(min(max(s, 2.0**-8), 2.0**14))


def _prepare(inputs):
    x = np.asarray(inputs["x"], dtype=np.float32)
    edge_attr = np.asarray(inputs["edge_attr"], dtype=np.float32)
    u = np.asarray(inputs["u"], dtype=np.float32)
    ei = np.asarray(inputs["edge_index"]).astype(np.int64)
    batch = np.asarray(inputs["batch"]).astype(np.int64)

    NN, D = x.shape
    NG = u.shape[0]
    NWIN = NCORES * NWPC

    src = ei[0]
    deg = np.bincount(src, minlength=NN).astype(np.float32)
    inv_deg = (1.0 / np.maximum(deg, 1.0)).astype(np.float32)
    cnt = np.bincount(batch, minlength=NG).astype(np.float32)
    inv_cnt = (1.0 / np.maximum(cnt, 1.0)).astype(np.float32)

    # nodes: sort by graph (setup_inputs already provides sorted batch)
    if np.any(batch[1:] < batch[:-1]):
        norder = np.argsort(batch, kind="stable")
        batch_s = batch[norder]
        x_s = x[norder]
    else:
        batch_s, x_s = batch, x

    gid = batch[src]
    eorder = np.argsort(gid, kind="stable")
    gid_s = gid[eorder]

    # fold both scatter_mean weight chains into the data, scale into fp8 range
    ea_w = edge_attr[eorder] * (inv_deg[src] * inv_cnt[gid])[eorder, None]
    se = _pow2_scale(ea_w)
    ea8 = (ea_w * se).astype(NPF8)
    x_w = x_s * inv_cnt[batch_s][:, None]
    sx = _pow2_scale(x_w)
    x8 = (x_w * sx).astype(NPF8)

    wstarts = np.arange(NWIN + 1) * WIN
    e_bnd = np.searchsorted(gid_s, wstarts)
    x_bnd = np.searchsorted(batch_s, wstarts)
    EW = max(_ceil_to(int((e_bnd[1:] - e_bnd[:-1]).max()), 2 * P), 2 * P)
    XW = max(_ceil_to(int((x_bnd[1:] - x_bnd[:-1]).max()), 2 * P), 2 * P)
    nt_e, nt_x = EW // P, XW // P

    w0t = np.ascontiguousarray(np.asarray(inputs["W0"], np.float16).T)
    w1t = np.ascontiguousarray(np.asarray(inputs["W1"], np.float16).T)
    w2t = np.ascontiguousarray(np.asarray(inputs["W2"], np.float16).T)
    par = np.ascontiguousarray(
        np.stack(
            [np.asarray(inputs[k], np.float32) for k in
             ("b0", "b1", "b2", "g0", "g1", "g2", "be0", "be1", "be2")],
            axis=1,
        )
    )

    def pack_core(c, data8, sorted_gid, bnd, nt, ch):
        """Chunk-major fp8 data (full + remainder chunks) + gid table."""
        nf, rem = nt // ch, nt % ch
        dat = np.zeros((NWPC * nf * P, ch * D), NPF8) if nf else None
        datr = np.zeros((NWPC * P, rem * D), NPF8) if rem else None
        g16 = np.full((P, NWPC * nt), -1.0, np.float16)
        for wi in range(NWPC):
            w = NWPC * c + wi
            lo, hi = int(bnd[w]), int(bnd[w + 1])
            n = hi - lo
            buf = np.zeros((nt * P, D), NPF8)
            buf[:n] = data8[lo:hi]
            if nf:
                dat[wi * nf * P : (wi + 1) * nf * P] = (
                    buf[: nf * ch * P]
                    .reshape(nf, ch, P, D).transpose(0, 2, 1, 3).reshape(nf * P, ch * D)
                )
            if rem:
                datr[wi * P : (wi + 1) * P] = (
                    buf[nf * ch * P :]
                    .reshape(rem, P, D).transpose(1, 0, 2).reshape(P, rem * D)
                )
            gl = np.full(nt * P, -1.0, np.float32)
            gl[:n] = sorted_gid[lo:hi] - w * WIN
            g16[:, wi * nt : (wi + 1) * nt] = gl.reshape(nt, P).T
        return dat, datr, g16

    gpc = NG // NCORES
    in_maps = []
    for c in range(NCORES):
        ea_c, ear_c, eg16 = pack_core(c, ea8, gid_s, e_bnd, nt_e, CHE)
        xa_c, xar_c, xg16 = pack_core(c, x8, batch_s, x_bnd, nt_x, CHX)
        m = {
            "eg16": eg16,
            "xg16": xg16,
            "ut": np.ascontiguousarray(u[c * gpc : (c + 1) * gpc].T.astype(np.float16)),
            "w0t": w0t, "w1t": w1t, "w2t": w2t, "par": par,
        }
        for k, v in (("ea", ea_c), ("ear", ear_c), ("xa", xa_c), ("xar", xar_c)):
            if v is not None:
                m[k] = v
        in_maps.append(m)

    cfg = {"D": D, "NG": NG, "EW": EW, "XW": XW, "se": se, "sx": sx}
    return cfg, in_maps


def kernel(**inputs) -> np.ndarray:
    cfg, in_maps = _prepare(inputs)
    nc = _build_program(cfg)
    res = bass_utils.run_bass_kernel_spmd(nc, in_maps, core_ids=list(range(NCORES)))
    return np.ascontiguousarray(
        np.concatenate([np.asarray(r["out_t"]).T for r in res.results], axis=0)
    ).astype(np.float32)


# revision 38
# speedup vs baseline: 1.0061x; 1.0061x over previous
"""Trainium2 Bass kernel: MEGNet GlobalModel (graph aggregation + 3-layer MLP w/ BatchNorm).

Strategy (graph-parallel over 8 NeuronCores):
  - 2048 graphs -> 64 windows of 32 graphs; core c owns windows 8c..8c+7.
  - Host folds the two chained scatter_means into the DATA itself:
        ea'[e] = edge_attr[e] * (1/max(deg[src_e],1)) * (1/max(cnt[g],1)) * S_e
        x'[n]  = x[n] * (1/max(cnt[g],1)) * S_x
    (S_* power-of-two scales keeping fp8 in range; undone at PSUM evacuation),
    sorts edges by graph id, and packs both streams chunk-major in fp8e4m3.
  - Device: per 256-row (2-tile) group, a 0/1 selection matrix sel[i, j] =
    (local_gid_i == j) covering the FULL 32-graph window is matmul'ed against
    the data in ONE DoubleRow fp8 matmul (2 edges per PE cell), accumulating
    per-graph sums in a [32, D] PSUM acc. Because sel spans the whole window,
    no slot bases / fixups / metadata are needed; the first pair of each
    window sets start=True which also zero-fills rows of empty graphs.
  - MLP is graph-sharded: each core runs the 3-layer MLP only for its own 256
    graphs in [feature, graph] layout. BatchNorm batch stats are the only
    cross-core coupling: per layer, a tiny [128, 4] f32 AllReduce of
    (sum, sumsq). A dummy warm-up collective at kernel start hides the cc
    firmware's first-launch latency. No AllGather of activations at all;
    the host concatenates the 8 per-core [D, 256] outputs.
"""

import sys

sys.path.insert(0, "/opt/trn_rl_repo")

import ml_dtypes
import numpy as np

from concourse import bacc, bass, bass_utils, mybir, tile
from concourse.masks import make_identity

F32 = mybir.dt.float32
F16 = mybir.dt.float16
F8 = mybir.dt.float8e4
NPF8 = ml_dtypes.float8_e4m3
P = 128
EPS = 1e-5
NCORES = 8
WIN = 32  # graphs per window (= sel width)
NWPC = 8  # windows per core
CHE = 32  # edge-stream [128, D] tiles per DMA chunk (1 MB chunks)
CHX = 16  # x-stream tiles per chunk
DR = mybir.MatmulPerfMode.DoubleRow
ALU = mybir.AluOpType
ACTF = mybir.ActivationFunctionType
AX = mybir.AxisListType
USE_TTR = False  # DVE tensor_tensor_reduce hangs the device on this stack

_prog_cache: dict = {}


def _ceil_to(a: int, m: int) -> int:
    return -(-a // m) * m


# ---------------------------------------------------------------- device program


def _emit(nc, tc, cfg, ap):
    D, NG, EW, XW = cfg["D"], cfg["NG"], cfg["EW"], cfg["XW"]
    nt_e, nt_x = EW // P, XW // P
    DJ = D // P  # feature tiles per 128 partitions (2)
    K1 = 3 * D // P  # k-tiles of layer 1 (6)
    GPC = NG // NCORES  # graphs per core (256)
    assert NG == NCORES * NWPC * WIN and D % P == 0

    with (
        tc.tile_pool(name="const", bufs=1) as cpool,
        tc.tile_pool(name="tables", bufs=1) as tpool,
        tc.tile_pool(name="data", bufs=6) as dpool,
        tc.tile_pool(name="eq", bufs=4) as qpool,
        tc.tile_pool(name="evac", bufs=2) as epool,
        tc.tile_pool(name="psum", bufs=2, space="PSUM") as ppool,
        tc.tile_pool(name="psumL1", bufs=1, space="PSUM") as pl1pool,
        tc.tile_pool(name="mlp", bufs=1) as mpool,
        tc.tile_pool(name="stats", bufs=2) as stpool,
        tc.tile_pool(name="dram", bufs=1, space="DRAM") as drpool,
    ):
        # --- constants
        ident = cpool.tile([P, P], F32)
        make_identity(nc, ident[:])
        iota_i = cpool.tile([P, WIN], mybir.dt.int32)
        nc.gpsimd.iota(iota_i[:], pattern=[[1, WIN]], base=0, channel_multiplier=0)
        iota16 = cpool.tile([P, WIN], F16)
        nc.vector.tensor_copy(iota16[:], iota_i[:])
        eps_sb = cpool.tile([P, 1], F32)
        nc.vector.memset(eps_sb[:], EPS)
        zero_sb = cpool.tile([P, 1], F32)
        nc.vector.memset(zero_sb[:], 0.0)
        iota3 = iota16[:, :].rearrange("p (o f) -> p o f", o=1)

        # --- per-row tables: window-local gid per tile column
        def table(name, cols):
            t = tpool.tile([P, cols], F16, name=name)
            nc.scalar.dma_start(t[:], ap[name][:, :])
            return t

        eg16 = table("eg16", NWPC * nt_e)
        xg16 = table("xg16", NWPC * nt_x)

        # --- MLP params, prefetched up front
        w0_sb = mpool.tile([P, K1, D], F16)
        nc.scalar.dma_start(w0_sb[:], ap["w0t"][:, :].rearrange("(a p) f -> p a f", p=P))
        w1_sb = mpool.tile([P, DJ, D], F16)
        w2_sb = mpool.tile([P, DJ, D], F16)
        par_sb = mpool.tile([P, DJ, 9], F32)
        nc.scalar.dma_start(par_sb[:], ap["par"][:, :].rearrange("(a p) c -> p a c", p=P))
        ut_sb = mpool.tile([P, DJ, GPC], F16)
        nc.scalar.dma_start(ut_sb[:], ap["ut"][:, :].rearrange("(a p) g -> p a g", p=P))

        # --- warm-up collectives (issued mid-stream; see main flow)
        warm_in = drpool.tile([P, 1], F32, name="warm_in")
        warm_outs = [
            drpool.tile([P, 1], F32, addr_space="Shared", name=f"warm_out{i}")
            for i in range(1)
        ]
        nc.sync.dma_start(warm_in[:], zero_sb[:])

        def warmup_collective(i):
            nc.gpsimd.collective_compute(
                "AllReduce",
                ALU.add,
                replica_groups=[list(range(NCORES))],
                ins=[warm_in.opt()],
                outs=[warm_outs[i].opt()],
            )

        # comb^T k-tiles per window: [ue0, ue1, uv0, uv1]; u comes from ut_sb
        combT = mpool.tile([P, NWPC, 4, WIN], F16, name="combT")

        # --- one window of segment-sum: acc[g, :] += sel.T @ rows (DoubleRow fp8)
        # chunk schedule: full CH-tile chunks + one per-window remainder chunk
        def seg_window(full_ap, rem_ap, g16, nt, win, inv_s, dst_k0, ch):
            acc = ppool.tile([WIN, 512], F32, tag="acc")  # full PSUM bank
            npair = nt // 2
            nfull, rem = nt // ch, nt % ch
            sched = [(full_ap, (win * nfull + c) * P, c * ch, ch) for c in range(nfull)]
            if rem:
                sched.append((rem_ap, win * P, nfull * ch, rem))
            for c, (src, r0, tile0, cw) in enumerate(sched):
                chunk = dpool.tile([P, cw, D], F8, tag="data")
                # alternate the two HWDGE rings to double descriptor throughput
                eng = nc.sync if c % 2 == 0 else nc.scalar
                eng.dma_start(chunk[:], src[r0 : r0 + P, :])
                cl, cr = win * nt + tile0, win * nt + tile0 + cw
                eq = qpool.tile([P, cw, WIN], F8, tag="eq")
                nc.vector.tensor_tensor(
                    out=eq[:],
                    in0=iota3.to_broadcast([P, cw, WIN]),
                    in1=g16[:, cl:cr].rearrange("p (c o) -> p c o", o=1).to_broadcast(
                        [P, cw, WIN]
                    ),
                    op=ALU.is_equal,
                )
                for s in range(cw // 2):
                    pr = tile0 // 2 + s
                    nc.tensor.matmul(
                        acc[:, 0:D],
                        lhsT=eq[:, 2 * s : 2 * s + 2, :],
                        rhs=chunk[:, 2 * s : 2 * s + 2, :],
                        start=(pr == 0),
                        stop=(pr == npair - 1),
                        perf_mode=DR,
                        skip_group_check=True,
                    )
            # evacuate: descale, transpose [WIN, 128] blocks into comb^T k-tiles
            acc_sb = epool.tile([WIN, D], F32, tag="acc_sb")
            nc.scalar.activation(acc_sb[:], acc[:, 0:D], ACTF.Copy, scale=inv_s)
            for fh in range(DJ):
                tp = ppool.tile([P, 512], F32, tag="mm")  # full PSUM bank
                nc.tensor.transpose(
                    tp[:, 0:WIN], acc_sb[:, fh * P : (fh + 1) * P], ident[0:WIN, 0:WIN]
                )
                nc.scalar.copy(combT[:, win, dst_k0 + fh, :], tp[:, 0:WIN])

        # --- graph-sharded MLP pieces
        psL1 = pl1pool.tile([P, DJ, NWPC, WIN], F32, name="psL1")
        h0 = mpool.tile([P, DJ, GPC], F16, name="h0")
        h1 = mpool.tile([P, DJ, GPC], F16, name="h1")
        hout = mpool.tile([P, DJ, GPC], F32, name="hout")
        st0 = mpool.tile([P, 2, DJ, NWPC], F32, name="st0")  # L1 (sum, sq) per window

        def l1_window(win):
            for jt in range(DJ):
                for kk in range(K1):
                    rhs = (
                        combT[:, win, kk, :]
                        if kk < 4
                        else ut_sb[:, kk - 4, win * WIN : (win + 1) * WIN]
                    )
                    nc.tensor.matmul(
                        psL1[:, jt, win, :],
                        lhsT=w0_sb[:, kk, jt * P : (jt + 1) * P],
                        rhs=rhs,
                        start=(kk == 0),
                        stop=(kk == K1 - 1),
                    )
                hsl = h0[:, jt, win * WIN : (win + 1) * WIN]
                nc.scalar.activation(
                    hsl,
                    psL1[:, jt, win, :],
                    ACTF.Relu,
                    bias=par_sb[:, jt, 0:1],
                    scale=1.0,
                    accum_out=st0[:, 0, jt, win : win + 1],
                )
                scr = stpool.tile([P, WIN], F16, tag="scr")
                nc.vector.tensor_tensor(out=scr[:], in0=hsl, in1=hsl, op=ALU.mult)
                nc.vector.tensor_reduce(
                    st0[:, 1, jt, win : win + 1], scr[:], axis=AX.X, op=ALU.add
                )

        def stats_allreduce(name, st_ap):
            # st_ap: [P, 2*DJ]-shaped f32 AP of local (sum, sumsq) partial
            sdr = drpool.tile([P, 2 * DJ], F32, name=f"stat{name}")
            gdr = drpool.tile([P, 2 * DJ], F32, addr_space="Shared", name=f"gstat{name}")
            nc.sync.dma_start(sdr[:], st_ap)
            nc.gpsimd.collective_compute(
                "AllReduce",
                ALU.add,
                replica_groups=[list(range(NCORES))],
                ins=[sdr.opt()],
                outs=[gdr.opt()],
            )
            return gdr

        def bn_finalize(layer, st_g):
            # st_g: [P, 2, DJ] f32 tile of GLOBAL (sum, sumsq) -> scl, bv
            me = stpool.tile([P, 2, DJ], F32, tag="me")
            nc.vector.tensor_scalar_mul(me[:], st_g[:], 1.0 / NG)
            mean, esq = me[:, 0, :], me[:, 1, :]
            m2 = stpool.tile([P, DJ], F32, tag="m2")
            nc.vector.tensor_tensor(out=m2[:], in0=mean, in1=mean, op=ALU.mult)
            var = stpool.tile([P, DJ], F32, tag="var")
            nc.vector.tensor_tensor(out=var[:], in0=esq, in1=m2[:], op=ALU.subtract)
            std = stpool.tile([P, DJ], F32, tag="std")
            nc.scalar.activation(std[:], var[:], ACTF.Sqrt, bias=eps_sb[:], scale=1.0)
            rstd = stpool.tile([P, DJ], F32, tag="rstd")
            nc.vector.reciprocal(rstd[:], std[:])
            scl = stpool.tile([P, DJ], F32, tag="scl")
            nc.vector.tensor_tensor(
                out=scl[:], in0=rstd[:], in1=par_sb[:, :, 3 + layer], op=ALU.mult
            )
            mscl = stpool.tile([P, DJ], F32, tag="mscl")
            nc.vector.tensor_tensor(out=mscl[:], in0=mean, in1=scl[:], op=ALU.mult)
            bv = stpool.tile([P, DJ], F32, tag="bv")
            nc.vector.tensor_tensor(
                out=bv[:], in0=par_sb[:, :, 6 + layer], in1=mscl[:], op=ALU.subtract
            )
            return scl, bv

        def bn_sync(layer, st_in):
            gdr = stats_allreduce(str(layer), st_in[:].rearrange("p a b -> p (a b)"))
            st_g = stpool.tile([P, 2, DJ], F32, tag="stg")
            nc.scalar.dma_start(st_g[:], gdr[:, :].rearrange("p (a b) -> p a b", b=DJ))
            return bn_finalize(layer, st_g)

        def apply_bn(h, scl, bv):
            for jt in range(DJ):
                nc.vector.tensor_scalar(
                    h[:, jt, :],
                    h[:, jt, :],
                    scalar1=scl[:, jt : jt + 1],
                    scalar2=bv[:, jt : jt + 1],
                    op0=ALU.mult,
                    op1=ALU.add,
                )

        def mid_layer(layer, w_sb, h_in, h_out):
            st_in = stpool.tile([P, 2, DJ], F32, tag="stin")
            ps = ppool.tile([P, DJ, GPC], F32, tag="mlp")
            for jt in range(DJ):
                for kk in range(DJ):
                    nc.tensor.matmul(
                        ps[:, jt, :],
                        lhsT=w_sb[:, kk, jt * P : (jt + 1) * P],
                        rhs=h_in[:, kk, :],
                        start=(kk == 0),
                        stop=(kk == DJ - 1),
                    )
                nc.scalar.activation(
                    h_out[:, jt, :],
                    ps[:, jt, :],
                    ACTF.Relu,
                    bias=par_sb[:, jt, layer : layer + 1],
                    scale=1.0,
                    accum_out=st_in[:, 0, jt : jt + 1],
                )
                scr = stpool.tile([P, GPC], F16, tag="scr")
                nc.vector.tensor_tensor(
                    out=scr[:], in0=h_out[:, jt, :], in1=h_out[:, jt, :], op=ALU.mult
                )
                nc.vector.tensor_reduce(
                    st_in[:, 1, jt : jt + 1], scr[:], axis=AX.X, op=ALU.add
                )
            return st_in

        # --- main flow
        for win in range(NWPC):
            seg_window(
                ap.get("xa"), ap.get("xar"), xg16, nt_x, win,
                1.0 / cfg["sx"], 2, CHX,
            )
            seg_window(
                ap.get("ea"), ap.get("ear"), eg16, nt_e, win,
                1.0 / cfg["se"], 0, CHE,
            )
            l1_window(win)
            if win == 0:
                # prime the collectives firmware so the BN AllReduces start hot
                warmup_collective(0)
                # L2/L3 weights aren't needed until the tail; load them behind
                # the first window's chunks so they don't delay the stream
                nc.scalar.dma_start(
                    w1_sb[:], ap["w1t"][:, :].rearrange("(a p) f -> p a f", p=P)
                )
                nc.scalar.dma_start(
                    w2_sb[:], ap["w2t"][:, :].rearrange("(a p) f -> p a f", p=P)
                )
            if win == NWPC - 2:
                # L1 stats of windows 0..NWPC-2: AllReduce overlaps the last
                # window's streaming; only the tiny last-window stats AR is
                # left on the post-stream critical path
                st_a = mpool.tile([P, 2, DJ], F32, name="st_a")
                nc.vector.tensor_reduce(
                    st_a[:], st0[:, :, :, : NWPC - 1], axis=AX.X, op=ALU.add
                )
                gdr0a = stats_allreduce("0a", st_a[:].rearrange("p a b -> p (a b)"))

        gdr0b = stats_allreduce(
            "0b", st0[:, :, :, NWPC - 1 : NWPC].rearrange("p a b o -> p (a b o)")
        )
        stg_a = stpool.tile([P, 2, DJ], F32, tag="stga")
        nc.scalar.dma_start(stg_a[:], gdr0a[:, :].rearrange("p (a b) -> p a b", b=DJ))
        stg_b = stpool.tile([P, 2, DJ], F32, tag="stg")
        nc.scalar.dma_start(stg_b[:], gdr0b[:, :].rearrange("p (a b) -> p a b", b=DJ))
        st_g0 = stpool.tile([P, 2, DJ], F32, tag="stg0")
        nc.vector.tensor_tensor(out=st_g0[:], in0=stg_a[:], in1=stg_b[:], op=ALU.add)
        scl, bv = bn_finalize(0, st_g0)
        apply_bn(h0, scl, bv)
        st_l2 = mid_layer(1, w1_sb, h0, h1)
        scl, bv = bn_sync(1, st_l2)
        apply_bn(h1, scl, bv)
        st_l3 = mid_layer(2, w2_sb, h1, hout)
        scl, bv = bn_sync(2, st_l3)
        apply_bn(hout, scl, bv)

        for jt in range(DJ):
            nc.sync.dma_start(ap["out_t"][jt * P : (jt + 1) * P, :], hout[:, jt, :])


def _build_program(cfg):
    key = repr(sorted(cfg.items(), key=lambda kv: kv[0]))
    if key in _prog_cache:
        return _prog_cache[key]
    D, NG, EW, XW = cfg["D"], cfg["NG"], cfg["EW"], cfg["XW"]
    nt_e, nt_x = EW // P, XW // P
    GPC = NG // NCORES
    nc = bacc.Bacc(
        "TRN2",
        target_bir_lowering=False,
        debug=False,
        enable_asserts=False,
        num_devices=NCORES,
    )
    ap = {}
    ins = [
        ("eg16", [P, NWPC * nt_e], F16),
        ("xg16", [P, NWPC * nt_x], F16),
        ("ut", [D, GPC], F16),
        ("w0t", [3 * D, D], F16),
        ("w1t", [D, D], F16),
        ("w2t", [D, D], F16),
        ("par", [D, 9], F32),
    ]
    for nt, ch, full, remn in ((nt_e, CHE, "ea", "ear"), (nt_x, CHX, "xa", "xar")):
        nf, rem = nt // ch, nt % ch
        if nf:
            ins.append((full, [NWPC * nf * P, ch * D], F8))
        if rem:
            ins.append((remn, [NWPC * P, rem * D], F8))
    for name, shape, dt in ins:
        ap[name] = nc.dram_tensor(name, shape, dt, kind="ExternalInput").ap()
    ap["out_t"] = nc.dram_tensor("out_t", [D, GPC], F32, kind="ExternalOutput").ap()

    with tile.TileContext(nc) as tc:
        _emit(nc, tc, cfg, ap)
    nc.compile()
    _prog_cache[key] = nc
    return nc


# ---------------------------------------------------------------- host side


def _pow2_scale(v: np.ndarray) -> float:
    m = float(np.max(np.abs(v))) if v.size else 0.0
    if not np.isfinite(m) or m <= 0.0:
        return 1.0
    s = 2.0 ** np.floor(np.log2(224.0 / m))
    return float(min(max(s, 2.0**-8), 2.0**14))


def _prepare(inputs):
    x = np.asarray(inputs["x"], dtype=np.float32)
    edge_attr = np.asarray(inputs["edge_attr"], dtype=np.float32)
    u = np.asarray(inputs["u"], dtype=np.float32)
    ei = np.asarray(inputs["edge_index"]).astype(np.int64)
    batch = np.asarray(inputs["batch"]).astype(np.int64)

    NN, D = x.shape
    NG = u.shape[0]
    NWIN = NCORES * NWPC

    src = ei[0]
    deg = np.bincount(src, minlength=NN).astype(np.float32)
    inv_deg = (1.0 / np.maximum(deg, 1.0)).astype(np.float32)
    cnt = np.bincount(batch, minlength=NG).astype(np.float32)
    inv_cnt = (1.0 / np.maximum(cnt, 1.0)).astype(np.float32)

    # nodes: sort by graph (setup_inputs already provides sorted batch)
    if np.any(batch[1:] < batch[:-1]):
        norder = np.argsort(batch, kind="stable")
        batch_s = batch[norder]
        x_s = x[norder]
    else:
        batch_s, x_s = batch, x

    gid = batch[src]
    eorder = np.argsort(gid, kind="stable")
    gid_s = gid[eorder]

    # fold both scatter_mean weight chains into the data, scale into fp8 range
    ea_w = edge_attr[eorder] * (inv_deg[src] * inv_cnt[gid])[eorder, None]
    se = _pow2_scale(ea_w)
    ea8 = (ea_w * se).astype(NPF8)
    x_w = x_s * inv_cnt[batch_s][:, None]
    sx = _pow2_scale(x_w)
    x8 = (x_w * sx).astype(NPF8)

    wstarts = np.arange(NWIN + 1) * WIN
    e_bnd = np.searchsorted(gid_s, wstarts)
    x_bnd = np.searchsorted(batch_s, wstarts)
    EW = max(_ceil_to(int((e_bnd[1:] - e_bnd[:-1]).max()), 2 * P), 2 * P)
    XW = max(_ceil_to(int((x_bnd[1:] - x_bnd[:-1]).max()), 2 * P), 2 * P)
    nt_e, nt_x = EW // P, XW // P

    w0t = np.ascontiguousarray(np.asarray(inputs["W0"], np.float16).T)
    w1t = np.ascontiguousarray(np.asarray(inputs["W1"], np.float16).T)
    w2t = np.ascontiguousarray(np.asarray(inputs["W2"], np.float16).T)
    par = np.ascontiguousarray(
        np.stack(
            [np.asarray(inputs[k], np.float32) for k in
             ("b0", "b1", "b2", "g0", "g1", "g2", "be0", "be1", "be2")],
            axis=1,
        )
    )

    def pack_core(c, data8, sorted_gid, bnd, nt, ch):
        """Chunk-major fp8 data (full + remainder chunks) + gid table."""
        nf, rem = nt // ch, nt % ch
        dat = np.zeros((NWPC * nf * P, ch * D), NPF8) if nf else None
        datr = np.zeros((NWPC * P, rem * D), NPF8) if rem else None
        g16 = np.full((P, NWPC * nt), -1.0, np.float16)
        for wi in range(NWPC):
            w = NWPC * c + wi
            lo, hi = int(bnd[w]), int(bnd[w + 1])
            n = hi - lo
            buf = np.zeros((nt * P, D), NPF8)
            buf[:n] = data8[lo:hi]
            if nf:
                dat[wi * nf * P : (wi + 1) * nf * P] = (
                    buf[: nf * ch * P]
                    .reshape(nf, ch, P, D).transpose(0, 2, 1, 3).reshape(nf * P, ch * D)
                )
            if rem:
                datr[wi * P : (wi + 1) * P] = (
                    buf[nf * ch * P :]
                    .reshape(rem, P, D).transpose(1, 0, 2).reshape(P, rem * D)
                )
            gl = np.full(nt * P, -1.0, np.float32)
            gl[:n] = sorted_gid[lo:hi] - w * WIN
            g16[:, wi * nt : (wi + 1) * nt] = gl.reshape(nt, P).T
        return dat, datr, g16

    gpc = NG // NCORES
    in_maps = []
    for c in range(NCORES):
        ea_c, ear_c, eg16 = pack_core(c, ea8, gid_s, e_bnd, nt_e, CHE)
        xa_c, xar_c, xg16 = pack_core(c, x8, batch_s, x_bnd, nt_x, CHX)
        m = {
            "eg16": eg16,
            "xg16": xg16,
            "ut": np.ascontiguousarray(u[c * gpc : (c + 1) * gpc].T.astype(np.float16)),
            "w0t": w0t, "w1t": w1t, "w2t": w2t, "par": par,
        }
        for k, v in (("ea", ea_c), ("ear", ear_c), ("xa", xa_c), ("xar", xar_c)):
            if v is not None:
                m[k] = v
        in_maps.append(m)

    cfg = {"D": D, "NG": NG, "EW": EW, "XW": XW, "se": se, "sx": sx}
    return cfg, in_maps


def kernel(**inputs) -> np.ndarray:
    cfg, in_maps = _prepare(inputs)
    nc = _build_program(cfg)
    res = bass_utils.run_bass_kernel_spmd(nc, in_maps, core_ids=list(range(NCORES)))
    return np.ascontiguousarray(
        np.concatenate([np.asarray(r["out_t"]).T for r in res.results], axis=0)
    ).astype(np.float32)


# revision 40
# speedup vs baseline: 1.0299x; 1.0237x over previous
"""Trainium2 Bass kernel: MEGNet GlobalModel (graph aggregation + 3-layer MLP w/ BatchNorm).

Strategy (graph-parallel over 8 NeuronCores):
  - 2048 graphs -> 64 windows of 32 graphs; core c owns windows 8c..8c+7.
  - Host folds the two chained scatter_means into the DATA itself:
        ea'[e] = edge_attr[e] * (1/max(deg[src_e],1)) * (1/max(cnt[g],1)) * S_e
        x'[n]  = x[n] * (1/max(cnt[g],1)) * S_x
    (S_* power-of-two scales keeping fp8 in range; undone at PSUM evacuation),
    sorts edges by graph id, and packs both streams chunk-major in fp8e4m3.
  - Device: per 256-row (2-tile) group, a 0/1 selection matrix sel[i, j] =
    (local_gid_i == j) covering the FULL 32-graph window is matmul'ed against
    the data in ONE DoubleRow fp8 matmul (2 edges per PE cell), accumulating
    per-graph sums in a [32, D] PSUM acc. Because sel spans the whole window,
    no slot bases / fixups / metadata are needed; the first pair of each
    window sets start=True which also zero-fills rows of empty graphs.
  - MLP is graph-sharded: each core runs the 3-layer MLP only for its own 256
    graphs in [feature, graph] layout. BatchNorm batch stats are the only
    cross-core coupling: per layer, a tiny [128, 4] f32 AllReduce of
    (sum, sumsq). A dummy warm-up collective at kernel start hides the cc
    firmware's first-launch latency. No AllGather of activations at all;
    the host concatenates the 8 per-core [D, 256] outputs.
"""

import sys

sys.path.insert(0, "/opt/trn_rl_repo")

import ml_dtypes
import numpy as np

from concourse import bacc, bass, bass_utils, mybir, tile
from concourse.masks import make_identity

F32 = mybir.dt.float32
F16 = mybir.dt.float16
F8 = mybir.dt.float8e4
NPF8 = ml_dtypes.float8_e4m3
P = 128
EPS = 1e-5
NCORES = 8
WIN = 32  # graphs per window (= sel width)
NWPC = 8  # windows per core
CHE = 32  # edge-stream [128, D] tiles per DMA chunk (1 MB chunks)
CHX = 16  # x-stream tiles per chunk
DR = mybir.MatmulPerfMode.DoubleRow
ALU = mybir.AluOpType
ACTF = mybir.ActivationFunctionType
AX = mybir.AxisListType
USE_TTR = False  # DVE tensor_tensor_reduce hangs the device on this stack

_prog_cache: dict = {}


def _ceil_to(a: int, m: int) -> int:
    return -(-a // m) * m


# ---------------------------------------------------------------- device program


def _emit(nc, tc, cfg, ap):
    D, NG, EW, XW = cfg["D"], cfg["NG"], cfg["EW"], cfg["XW"]
    nt_e, nt_x = EW // P, XW // P
    DJ = D // P  # feature tiles per 128 partitions (2)
    K1 = 3 * D // P  # k-tiles of layer 1 (6)
    GPC = NG // NCORES  # graphs per core (256)
    assert NG == NCORES * NWPC * WIN and D % P == 0

    with (
        tc.tile_pool(name="const", bufs=1) as cpool,
        tc.tile_pool(name="tables", bufs=1) as tpool,
        tc.tile_pool(name="data", bufs=6) as dpool,
        tc.tile_pool(name="eq", bufs=4) as qpool,
        tc.tile_pool(name="evac", bufs=2) as epool,
        tc.tile_pool(name="psum", bufs=2, space="PSUM") as ppool,
        tc.tile_pool(name="psumL1", bufs=1, space="PSUM") as pl1pool,
        tc.tile_pool(name="mlp", bufs=1) as mpool,
        tc.tile_pool(name="stats", bufs=2) as stpool,
        tc.tile_pool(name="dram", bufs=1, space="DRAM") as drpool,
    ):
        # --- constants
        ident = cpool.tile([P, P], F32)
        make_identity(nc, ident[:])
        iota_i = cpool.tile([P, WIN], mybir.dt.int32)
        nc.gpsimd.iota(iota_i[:], pattern=[[1, WIN]], base=0, channel_multiplier=0)
        iota16 = cpool.tile([P, WIN], F16)
        nc.vector.tensor_copy(iota16[:], iota_i[:])
        eps_sb = cpool.tile([P, 1], F32)
        nc.vector.memset(eps_sb[:], EPS)
        zero_sb = cpool.tile([P, 1], F32)
        nc.vector.memset(zero_sb[:], 0.0)
        iota3 = iota16[:, :].rearrange("p (o f) -> p o f", o=1)

        # --- per-row tables: window-local gid per tile column
        def table(name, cols):
            t = tpool.tile([P, cols], F16, name=name)
            nc.scalar.dma_start(t[:], ap[name][:, :])
            return t

        eg16 = table("eg16", NWPC * nt_e)
        xg16 = table("xg16", NWPC * nt_x)

        # --- MLP params, prefetched up front
        w0_sb = mpool.tile([P, K1, D], F16)
        nc.scalar.dma_start(w0_sb[:], ap["w0t"][:, :].rearrange("(a p) f -> p a f", p=P))
        w1_sb = mpool.tile([P, DJ, D], F16)
        w2_sb = mpool.tile([P, DJ, D], F16)
        par_sb = mpool.tile([P, DJ, 9], F32)
        nc.scalar.dma_start(par_sb[:], ap["par"][:, :].rearrange("(a p) c -> p a c", p=P))
        ut_sb = mpool.tile([P, DJ, GPC], F16)
        nc.scalar.dma_start(ut_sb[:], ap["ut"][:, :].rearrange("(a p) g -> p a g", p=P))

        # --- warm-up collectives (issued mid-stream; see main flow)
        warm_in = drpool.tile([P, 1], F32, name="warm_in")
        warm_outs = [
            drpool.tile([P, 1], F32, addr_space="Shared", name=f"warm_out{i}")
            for i in range(1)
        ]
        nc.sync.dma_start(warm_in[:], zero_sb[:])

        def warmup_collective(i):
            nc.gpsimd.collective_compute(
                "AllReduce",
                ALU.add,
                replica_groups=[list(range(NCORES))],
                ins=[warm_in.opt()],
                outs=[warm_outs[i].opt()],
            )

        # comb^T k-tiles per window: [ue0, ue1, uv0, uv1]; u comes from ut_sb
        combT = mpool.tile([P, NWPC, 4, WIN], F16, name="combT")

        # --- one window of segment-sum: acc[g, :] += sel.T @ rows (DoubleRow fp8)
        # chunk schedule: full CH-tile chunks + one per-window remainder chunk
        def seg_window(full_ap, rem_ap, g16, nt, win, inv_s, dst_k0, ch):
            acc = ppool.tile([WIN, 512], F32, tag="acc")  # full PSUM bank
            npair = nt // 2
            nfull, rem = nt // ch, nt % ch
            sched = [(full_ap, (win * nfull + c) * P, c * ch, ch) for c in range(nfull)]
            if rem:
                sched.append((rem_ap, win * P, nfull * ch, rem))
            for c, (src, r0, tile0, cw) in enumerate(sched):
                chunk = dpool.tile([P, cw, D], F8, tag="data")
                # alternate the two HWDGE rings to double descriptor throughput
                eng = nc.sync if c % 2 == 0 else nc.scalar
                eng.dma_start(chunk[:], src[r0 : r0 + P, :])
                cl, cr = win * nt + tile0, win * nt + tile0 + cw
                eq = qpool.tile([P, cw, WIN], F8, tag="eq")
                nc.vector.tensor_tensor(
                    out=eq[:],
                    in0=iota3.to_broadcast([P, cw, WIN]),
                    in1=g16[:, cl:cr].rearrange("p (c o) -> p c o", o=1).to_broadcast(
                        [P, cw, WIN]
                    ),
                    op=ALU.is_equal,
                )
                for s in range(cw // 2):
                    pr = tile0 // 2 + s
                    nc.tensor.matmul(
                        acc[:, 0:D],
                        lhsT=eq[:, 2 * s : 2 * s + 2, :],
                        rhs=chunk[:, 2 * s : 2 * s + 2, :],
                        start=(pr == 0),
                        stop=(pr == npair - 1),
                        perf_mode=DR,
                        skip_group_check=True,
                    )
            # evacuate: descale, transpose [WIN, 128] blocks into comb^T k-tiles
            acc_sb = epool.tile([WIN, D], F32, tag="acc_sb")
            nc.scalar.activation(acc_sb[:], acc[:, 0:D], ACTF.Copy, scale=inv_s)
            for fh in range(DJ):
                tp = ppool.tile([P, 512], F32, tag="mm")  # full PSUM bank
                nc.tensor.transpose(
                    tp[:, 0:WIN], acc_sb[:, fh * P : (fh + 1) * P], ident[0:WIN, 0:WIN]
                )
                nc.scalar.copy(combT[:, win, dst_k0 + fh, :], tp[:, 0:WIN])

        # --- graph-sharded MLP pieces
        psL1 = pl1pool.tile([P, DJ, NWPC, WIN], F32, name="psL1")
        h0 = mpool.tile([P, DJ, GPC], F16, name="h0")
        h1 = mpool.tile([P, DJ, GPC], F16, name="h1")
        hout = mpool.tile([P, DJ, GPC], F32, name="hout")
        st0 = mpool.tile([P, 2, DJ, NWPC], F32, name="st0")  # L1 (sum, sq) per window

        def l1_window(win):
            for jt in range(DJ):
                for kk in range(K1):
                    rhs = (
                        combT[:, win, kk, :]
                        if kk < 4
                        else ut_sb[:, kk - 4, win * WIN : (win + 1) * WIN]
                    )
                    nc.tensor.matmul(
                        psL1[:, jt, win, :],
                        lhsT=w0_sb[:, kk, jt * P : (jt + 1) * P],
                        rhs=rhs,
                        start=(kk == 0),
                        stop=(kk == K1 - 1),
                    )
                hsl = h0[:, jt, win * WIN : (win + 1) * WIN]
                nc.scalar.activation(
                    hsl,
                    psL1[:, jt, win, :],
                    ACTF.Relu,
                    bias=par_sb[:, jt, 0:1],
                    scale=1.0,
                    accum_out=st0[:, 0, jt, win : win + 1],
                )
                scr = stpool.tile([P, WIN], F16, tag="scr")
                nc.vector.tensor_tensor(out=scr[:], in0=hsl, in1=hsl, op=ALU.mult)
                nc.vector.tensor_reduce(
                    st0[:, 1, jt, win : win + 1], scr[:], axis=AX.X, op=ALU.add
                )

        def stats_allreduce(name, st_ap):
            # st_ap: [P, 2*DJ]-shaped f32 AP of local (sum, sumsq) partial.
            # SWDGE (gpsimd) ring: a sem-wait here must not block the
            # HWDGE rings that carry the data stream.
            sdr = drpool.tile([P, 2 * DJ], F32, name=f"stat{name}")
            gdr = drpool.tile([P, 2 * DJ], F32, addr_space="Shared", name=f"gstat{name}")
            nc.gpsimd.dma_start(sdr[:], st_ap)
            nc.gpsimd.collective_compute(
                "AllReduce",
                ALU.add,
                replica_groups=[list(range(NCORES))],
                ins=[sdr.opt()],
                outs=[gdr.opt()],
            )
            return gdr

        def bn_finalize(layer, st_g):
            # st_g: [P, 2, DJ] f32 tile of GLOBAL (sum, sumsq) -> scl, bv
            me = stpool.tile([P, 2, DJ], F32, tag="me")
            nc.vector.tensor_scalar_mul(me[:], st_g[:], 1.0 / NG)
            mean, esq = me[:, 0, :], me[:, 1, :]
            m2 = stpool.tile([P, DJ], F32, tag="m2")
            nc.vector.tensor_tensor(out=m2[:], in0=mean, in1=mean, op=ALU.mult)
            var = stpool.tile([P, DJ], F32, tag="var")
            nc.vector.tensor_tensor(out=var[:], in0=esq, in1=m2[:], op=ALU.subtract)
            std = stpool.tile([P, DJ], F32, tag="std")
            nc.scalar.activation(std[:], var[:], ACTF.Sqrt, bias=eps_sb[:], scale=1.0)
            rstd = stpool.tile([P, DJ], F32, tag="rstd")
            nc.vector.reciprocal(rstd[:], std[:])
            scl = stpool.tile([P, DJ], F32, tag="scl")
            nc.vector.tensor_tensor(
                out=scl[:], in0=rstd[:], in1=par_sb[:, :, 3 + layer], op=ALU.mult
            )
            mscl = stpool.tile([P, DJ], F32, tag="mscl")
            nc.vector.tensor_tensor(out=mscl[:], in0=mean, in1=scl[:], op=ALU.mult)
            bv = stpool.tile([P, DJ], F32, tag="bv")
            nc.vector.tensor_tensor(
                out=bv[:], in0=par_sb[:, :, 6 + layer], in1=mscl[:], op=ALU.subtract
            )
            return scl, bv

        def bn_sync(layer, st_in):
            gdr = stats_allreduce(str(layer), st_in[:].rearrange("p a b -> p (a b)"))
            st_g = stpool.tile([P, 2, DJ], F32, tag="stg")
            nc.scalar.dma_start(st_g[:], gdr[:, :].rearrange("p (a b) -> p a b", b=DJ))
            return bn_finalize(layer, st_g)

        def apply_bn(h, scl, bv):
            for jt in range(DJ):
                nc.vector.tensor_scalar(
                    h[:, jt, :],
                    h[:, jt, :],
                    scalar1=scl[:, jt : jt + 1],
                    scalar2=bv[:, jt : jt + 1],
                    op0=ALU.mult,
                    op1=ALU.add,
                )

        def mid_layer(layer, w_sb, h_in, h_out):
            st_in = stpool.tile([P, 2, DJ], F32, tag="stin")
            ps = ppool.tile([P, DJ, GPC], F32, tag="mlp")
            for jt in range(DJ):
                for kk in range(DJ):
                    nc.tensor.matmul(
                        ps[:, jt, :],
                        lhsT=w_sb[:, kk, jt * P : (jt + 1) * P],
                        rhs=h_in[:, kk, :],
                        start=(kk == 0),
                        stop=(kk == DJ - 1),
                    )
                nc.scalar.activation(
                    h_out[:, jt, :],
                    ps[:, jt, :],
                    ACTF.Relu,
                    bias=par_sb[:, jt, layer : layer + 1],
                    scale=1.0,
                    accum_out=st_in[:, 0, jt : jt + 1],
                )
                scr = stpool.tile([P, GPC], F16, tag="scr")
                nc.vector.tensor_tensor(
                    out=scr[:], in0=h_out[:, jt, :], in1=h_out[:, jt, :], op=ALU.mult
                )
                nc.vector.tensor_reduce(
                    st_in[:, 1, jt : jt + 1], scr[:], axis=AX.X, op=ALU.add
                )
            return st_in

        # --- main flow
        for win in range(NWPC):
            seg_window(
                ap.get("xa"), ap.get("xar"), xg16, nt_x, win,
                1.0 / cfg["sx"], 2, CHX,
            )
            seg_window(
                ap.get("ea"), ap.get("ear"), eg16, nt_e, win,
                1.0 / cfg["se"], 0, CHE,
            )
            l1_window(win)
            if win == 0:
                # prime the collectives firmware so the BN AllReduces start hot
                warmup_collective(0)
                # L2/L3 weights aren't needed until the tail; load them behind
                # the first window's chunks so they don't delay the stream
                nc.scalar.dma_start(
                    w1_sb[:], ap["w1t"][:, :].rearrange("(a p) f -> p a f", p=P)
                )
                nc.scalar.dma_start(
                    w2_sb[:], ap["w2t"][:, :].rearrange("(a p) f -> p a f", p=P)
                )
            if win == NWPC - 3:
                # L1 stats of windows 0..NWPC-3: AllReduce overlaps the last
                # two windows' streaming; only the tiny tail-window stats AR
                # is left on the post-stream critical path
                st_a = mpool.tile([P, 2, DJ], F32, name="st_a")
                nc.vector.tensor_reduce(
                    st_a[:], st0[:, :, :, : NWPC - 2], axis=AX.X, op=ALU.add
                )
                gdr0a = stats_allreduce("0a", st_a[:].rearrange("p a b -> p (a b)"))

        st_b = mpool.tile([P, 2, DJ], F32, name="st_b")
        nc.vector.tensor_reduce(
            st_b[:], st0[:, :, :, NWPC - 2 :], axis=AX.X, op=ALU.add
        )
        gdr0b = stats_allreduce("0b", st_b[:].rearrange("p a b -> p (a b)"))
        stg_a = stpool.tile([P, 2, DJ], F32, tag="stga")
        nc.scalar.dma_start(stg_a[:], gdr0a[:, :].rearrange("p (a b) -> p a b", b=DJ))
        stg_b = stpool.tile([P, 2, DJ], F32, tag="stg")
        nc.scalar.dma_start(stg_b[:], gdr0b[:, :].rearrange("p (a b) -> p a b", b=DJ))
        st_g0 = stpool.tile([P, 2, DJ], F32, tag="stg0")
        nc.vector.tensor_tensor(out=st_g0[:], in0=stg_a[:], in1=stg_b[:], op=ALU.add)
        scl, bv = bn_finalize(0, st_g0)
        apply_bn(h0, scl, bv)
        st_l2 = mid_layer(1, w1_sb, h0, h1)
        scl, bv = bn_sync(1, st_l2)
        apply_bn(h1, scl, bv)
        st_l3 = mid_layer(2, w2_sb, h1, hout)
        scl, bv = bn_sync(2, st_l3)
        apply_bn(hout, scl, bv)

        for jt in range(DJ):
            nc.sync.dma_start(ap["out_t"][jt * P : (jt + 1) * P, :], hout[:, jt, :])


def _build_program(cfg):
    key = repr(sorted(cfg.items(), key=lambda kv: kv[0]))
    if key in _prog_cache:
        return _prog_cache[key]
    D, NG, EW, XW = cfg["D"], cfg["NG"], cfg["EW"], cfg["XW"]
    nt_e, nt_x = EW // P, XW // P
    GPC = NG // NCORES
    nc = bacc.Bacc(
        "TRN2",
        target_bir_lowering=False,
        debug=False,
        enable_asserts=False,
        num_devices=NCORES,
    )
    ap = {}
    ins = [
        ("eg16", [P, NWPC * nt_e], F16),
        ("xg16", [P, NWPC * nt_x], F16),
        ("ut", [D, GPC], F16),
        ("w0t", [3 * D, D], F16),
        ("w1t", [D, D], F16),
        ("w2t", [D, D], F16),
        ("par", [D, 9], F32),
    ]
    for nt, ch, full, remn in ((nt_e, CHE, "ea", "ear"), (nt_x, CHX, "xa", "xar")):
        nf, rem = nt // ch, nt % ch
        if nf:
            ins.append((full, [NWPC * nf * P, ch * D], F8))
        if rem:
            ins.append((remn, [NWPC * P, rem * D], F8))
    for name, shape, dt in ins:
        ap[name] = nc.dram_tensor(name, shape, dt, kind="ExternalInput").ap()
    ap["out_t"] = nc.dram_tensor("out_t", [D, GPC], F32, kind="ExternalOutput").ap()

    with tile.TileContext(nc) as tc:
        _emit(nc, tc, cfg, ap)
    nc.compile()
    _prog_cache[key] = nc
    return nc


# ---------------------------------------------------------------- host side


def _pow2_scale(v: np.ndarray) -> float:
    m = float(np.max(np.abs(v))) if v.size else 0.0
    if not np.isfinite(m) or m <= 0.0:
        return 1.0
    s = 2.0 ** np.floor(np.log2(224.0 / m))
    return float(min(max(s, 2.0**-8), 2.0**14))


def _prepare(inputs):
    x = np.asarray(inputs["x"], dtype=np.float32)
    edge_attr = np.asarray(inputs["edge_attr"], dtype=np.float32)
    u = np.asarray(inputs["u"], dtype=np.float32)
    ei = np.asarray(inputs["edge_index"]).astype(np.int64)
    batch = np.asarray(inputs["batch"]).astype(np.int64)

    NN, D = x.shape
    NG = u.shape[0]
    NWIN = NCORES * NWPC

    src = ei[0]
    deg = np.bincount(src, minlength=NN).astype(np.float32)
    inv_deg = (1.0 / np.maximum(deg, 1.0)).astype(np.float32)
    cnt = np.bincount(batch, minlength=NG).astype(np.float32)
    inv_cnt = (1.0 / np.maximum(cnt, 1.0)).astype(np.float32)

    # nodes: sort by graph (setup_inputs already provides sorted batch)
    if np.any(batch[1:] < batch[:-1]):
        norder = np.argsort(batch, kind="stable")
        batch_s = batch[norder]
        x_s = x[norder]
    else:
        batch_s, x_s = batch, x

    gid = batch[src]
    eorder = np.argsort(gid, kind="stable")
    gid_s = gid[eorder]

    # fold both scatter_mean weight chains into the data, scale into fp8 range
    ea_w = edge_attr[eorder] * (inv_deg[src] * inv_cnt[gid])[eorder, None]
    se = _pow2_scale(ea_w)
    ea8 = (ea_w * se).astype(NPF8)
    x_w = x_s * inv_cnt[batch_s][:, None]
    sx = _pow2_scale(x_w)
    x8 = (x_w * sx).astype(NPF8)

    wstarts = np.arange(NWIN + 1) * WIN
    e_bnd = np.searchsorted(gid_s, wstarts)
    x_bnd = np.searchsorted(batch_s, wstarts)
    EW = max(_ceil_to(int((e_bnd[1:] - e_bnd[:-1]).max()), 2 * P), 2 * P)
    XW = max(_ceil_to(int((x_bnd[1:] - x_bnd[:-1]).max()), 2 * P), 2 * P)
    nt_e, nt_x = EW // P, XW // P

    w0t = np.ascontiguousarray(np.asarray(inputs["W0"], np.float16).T)
    w1t = np.ascontiguousarray(np.asarray(inputs["W1"], np.float16).T)
    w2t = np.ascontiguousarray(np.asarray(inputs["W2"], np.float16).T)
    par = np.ascontiguousarray(
        np.stack(
            [np.asarray(inputs[k], np.float32) for k in
             ("b0", "b1", "b2", "g0", "g1", "g2", "be0", "be1", "be2")],
            axis=1,
        )
    )

    def pack_core(c, data8, sorted_gid, bnd, nt, ch):
        """Chunk-major fp8 data (full + remainder chunks) + gid table."""
        nf, rem = nt // ch, nt % ch
        dat = np.zeros((NWPC * nf * P, ch * D), NPF8) if nf else None
        datr = np.zeros((NWPC * P, rem * D), NPF8) if rem else None
        g16 = np.full((P, NWPC * nt), -1.0, np.float16)
        for wi in range(NWPC):
            w = NWPC * c + wi
            lo, hi = int(bnd[w]), int(bnd[w + 1])
            n = hi - lo
            buf = np.zeros((nt * P, D), NPF8)
            buf[:n] = data8[lo:hi]
            if nf:
                dat[wi * nf * P : (wi + 1) * nf * P] = (
                    buf[: nf * ch * P]
                    .reshape(nf, ch, P, D).transpose(0, 2, 1, 3).reshape(nf * P, ch * D)
                )
            if rem:
                datr[wi * P : (wi + 1) * P] = (
                    buf[nf * ch * P :]
                    .reshape(rem, P, D).transpose(1, 0, 2).reshape(P, rem * D)
                )
            gl = np.full(nt * P, -1.0, np.float32)
            gl[:n] = sorted_gid[lo:hi] - w * WIN
            g16[:, wi * nt : (wi + 1) * nt] = gl.reshape(nt, P).T
        return dat, datr, g16

    gpc = NG // NCORES
    in_maps = []
    for c in range(NCORES):
        ea_c, ear_c, eg16 = pack_core(c, ea8, gid_s, e_bnd, nt_e, CHE)
        xa_c, xar_c, xg16 = pack_core(c, x8, batch_s, x_bnd, nt_x, CHX)
        m = {
            "eg16": eg16,
            "xg16": xg16,
            "ut": np.ascontiguousarray(u[c * gpc : (c + 1) * gpc].T.astype(np.float16)),
            "w0t": w0t, "w1t": w1t, "w2t": w2t, "par": par,
        }
        for k, v in (("ea", ea_c), ("ear", ear_c), ("xa", xa_c), ("xar", xar_c)):
            if v is not None:
                m[k] = v
        in_maps.append(m)

    cfg = {"D": D, "NG": NG, "EW": EW, "XW": XW, "se": se, "sx": sx}
    return cfg, in_maps


def kernel(**inputs) -> np.ndarray:
    cfg, in_maps = _prepare(inputs)
    nc = _build_program(cfg)
    res = bass_utils.run_bass_kernel_spmd(nc, in_maps, core_ids=list(range(NCORES)))
    return np.ascontiguousarray(
        np.concatenate([np.asarray(r["out_t"]).T for r in res.results], axis=0)
    ).astype(np.float32)
